# revision 1
# baseline (speedup 1.0000x reference)
"""CascadeHadamardSmoothLinear Trainium2 kernel.

out = Q_nvfp4(hadamard_rotate(x * smooth_scale * S_in)) @ W.T + bias

Sharding: data-parallel over batch*seq rows across 8 cores; all other
tensors replicated.  Per core:
  - fold smooth_scale*S_in into per-block Hadamard matrices H'
  - PE-transpose x blocks, rotate via fp32 matmul (lhsT=x^T, rhs=H'_b)
  - NVFP4 fake-quant on DVE/ACT/GPSIMD (exact snap-to-level arithmetic:
    level2 = rint(min(y2,4.5)) + bitround(max(y2,4)) - 4, y2 = 12|x|/amax)
  - stream W with f32->bf16 cast during DMA (SWDGE), PE-transpose W tiles,
    main matmul in bf16 with f32 PSUM accumulation, + bias.
Prep is emitted chunk-major so the main matmul's k-accumulation can start
while later chunks are still quantizing.
"""
from contextlib import ExitStack

import numpy as np

F32_M_MAGIC = 12582912.0  # 1.5 * 2**23: add+sub rounds f32 to int (RNE)

_CACHE = {}


def _build(rows, din, dout, repeat=1):
    """Build the per-core Bass program. Same program on all cores (SPMD)."""
    import concourse.bass as bass
    import concourse.tile as tile
    from concourse import bacc, masks, mybir
    from concourse.alu_op_type import AluOpType as ALU

    F32 = mybir.dt.float32
    I32 = mybir.dt.int32
    BF16 = mybir.dt.bfloat16
    AX = mybir.AxisListType

    HB = 128                 # hadamard block
    NB = din // HB           # k-blocks (32)
    NM = rows // 128         # m-tiles (4)
    NN = dout // 512         # n-tiles (8)
    NC = din // 512          # quant chunks (4 blocks each) (8)
    KH = din // 2            # half of k, for W load chunking
    BITS4 = int(np.float32(4.0).view(np.int32))

    nc = bacc.Bacc("TRN2", target_bir_lowering=False, debug=False)
    x_d = nc.dram_tensor("x", (rows, din), F32, kind="ExternalInput")
    ss_d = nc.dram_tensor("smooth", (din,), F32, kind="ExternalInput")
    si_d = nc.dram_tensor("sin", (din,), F32, kind="ExternalInput")
    h_d = nc.dram_tensor("hb", (HB, HB), F32, kind="ExternalInput")
    w_d = nc.dram_tensor("w", (dout, din), F32, kind="ExternalInput")
    b_d = nc.dram_tensor("bias", (dout,), F32, kind="ExternalInput")
    o_d = nc.dram_tensor("out", (rows, dout), F32, kind="ExternalOutput")

    with tile.TileContext(nc) as tc, ExitStack() as ctx:
        cpool = ctx.enter_context(tc.tile_pool(name="const", bufs=1))
        ident_f = cpool.tile([128, 128], F32)
        masks.make_identity(nc, ident_f[:])
        ident_b = cpool.tile([128, 128], BF16)
        masks.make_identity(nc, ident_b[:])

        # s = smooth * S_in laid out [128 (k within block), NB (block)]:
        # load both naturally as [NB, 128], multiply, PE-transpose once.
        sst = cpool.tile([NB, HB], F32)
        sit = cpool.tile([NB, HB], F32)
        nc.sync.dma_start(sst[:], ss_d[:].rearrange("(b p) -> b p", p=HB))
        nc.sync.dma_start(sit[:], si_d[:].rearrange("(b p) -> b p", p=HB))
        svn = cpool.tile([NB, HB], F32)
        nc.vector.tensor_tensor(svn[:], sst[:], sit[:], op=ALU.mult)
        sv = cpool.tile([128, NB], F32)
        with tc.tile_pool(name="sv_ps", bufs=1, space="PSUM") as svps_pool:
            svp = svps_pool.tile([128, NB], F32)
            nc.tensor.transpose(svp[:], svn[:], ident_f[:NB, :NB])
            nc.scalar.copy(sv[:], svp[:])

        # H'[k, b*128+l] = H[k, l] * s[k, b]  (per-partition scalar multiply)
        ht = cpool.tile([128, HB], F32)
        nc.sync.dma_start(ht[:], h_d[:, :])
        hp = cpool.tile([128, NB * HB], F32)
        for b in range(NB):
            nc.vector.tensor_scalar(
                hp[:, b * HB : (b + 1) * HB], ht[:], sv[:, b : b + 1], None, op0=ALU.mult
            )

        # bias broadcast to all partitions
        bt = cpool.tile([128, dout], F32)
        nc.sync.dma_start(bt[:], b_d[:].unsqueeze(0).broadcast_to((128, dout)))

        # per-block quantized-transposed activations: xqt[b] is
        # [128 (k in block), NM*128 (m)] bf16, written by prep chunk b//4,
        # read by every n-tile's matmuls.
        xqt = [
            cpool.tile([128, NM * 128], BF16, name=f"xqt{b}", tag=f"xqt{b}")
            for b in range(NB)
        ]

        def _emit_body():
            with (
                tc.tile_pool(name="xload", bufs=2) as xpool,
                tc.tile_pool(name="tpsum", bufs=3, space="PSUM") as tps_pool,
                tc.tile_pool(name="xt_sb", bufs=5) as xts_pool,
                tc.tile_pool(name="rot_ps", bufs=3, space="PSUM") as rot_pool,
                tc.tile_pool(name="q", bufs=2) as qpool,
                tc.tile_pool(name="qs", bufs=2) as qspool,
                tc.tile_pool(name="xqt_ps", bufs=2, space="PSUM") as xqtps_pool,
            ):
                # ---------------- PREP: rotate + quantize + transpose ----------
                for c in range(NC):
                    for m in range(NM):
                        xc = xpool.tile([128, 512], F32, name=f"xc{m}", tag=f"xc{m}")
                        nc.sync.dma_start(
                            xc[:], x_d[m * 128 : (m + 1) * 128, c * 512 : (c + 1) * 512]
                        )
                        rps = rot_pool.tile([128, 512], F32, name="rps", tag="rps")
                        for jj in range(4):
                            b = 4 * c + jj
                            tps = tps_pool.tile([128, 128], F32, name="tps", tag="tps")
                            nc.tensor.transpose(
                                tps[:], xc[:, jj * HB : (jj + 1) * HB], ident_f[:]
                            )
                            tsb = xts_pool.tile([128, 128], F32, name="tsb", tag="tsb")
                            nc.scalar.copy(tsb[:], tps[:])
                            nc.tensor.matmul(
                                rps[:, jj * 128 : (jj + 1) * 128],
                                tsb[:],
                                hp[:, b * HB : (b + 1) * HB],
                                start=True,
                                stop=True,
                            )
                        # ---- NVFP4 quant of rps [128, 512] (16-groups on free dim)
                        r3 = rps[:].rearrange("p (g s) -> p g s", s=16)
                        amax = qspool.tile([128, 32], F32, name="amax", tag="amax")
                        nc.vector.tensor_reduce(
                            amax[:], r3, axis=AX.X, op=ALU.max, apply_absolute_value=True
                        )
                        s12 = qspool.tile([128, 32], F32, name="s12", tag="s12")
                        nc.vector.tensor_scalar(
                            s12[:], amax[:], 1.0 / 12.0, 1e-12 / 12.0, op0=ALU.mult, op1=ALU.max
                        )
                        inv12 = qspool.tile([128, 32], F32, name="inv12", tag="inv12")
                        nc.vector.reciprocal(inv12[:], s12[:])
                        inv_bc = inv12[:].unsqueeze(2).broadcast_to((128, 32, 16))
                        s12_bc = s12[:].unsqueeze(2).broadcast_to((128, 32, 16))

                        ys2 = qpool.tile([128, 512], F32, name="ys2", tag="ys2")
                        nc.vector.tensor_tensor(
                            ys2[:].rearrange("p (g s) -> p g s", s=16), r3, inv_bc, op=ALU.mult
                        )
                        y2 = qpool.tile([128, 512], F32, name="y2", tag="y2")
                        nc.vector.tensor_scalar(
                            y2[:].bitcast(I32), ys2[:].bitcast(I32), 0x7FFFFFFF, None,
                            op0=ALU.bitwise_and,
                        )
                        sgn = qpool.tile([128, 512], F32, name="sgn", tag="sgn")
                        nc.scalar.sign(sgn[:], ys2[:])
                        # A-branch: e = min(y2, 4.5) + M
                        e = qpool.tile([128, 512], F32, name="e", tag="e")
                        nc.vector.tensor_scalar(
                            e[:], y2[:], 4.5, F32_M_MAGIC, op0=ALU.min, op1=ALU.add
                        )
                        # B-branch (int): bitround(max(y2, 4.0))
                        g1 = qpool.tile([128, 512], I32, name="g1", tag="g1")
                        nc.vector.tensor_scalar(
                            g1[:], y2[:].bitcast(I32), BITS4, 0x00200000, op0=ALU.max, op1=ALU.add
                        )
                        b2 = qpool.tile([128, 512], F32, name="b2", tag="b2")
                        nc.vector.tensor_scalar(
                            b2[:].bitcast(I32), g1[:], -0x400000, None, op0=ALU.bitwise_and
                        )
                        # r2m4 = (e - (M+4)) + b2 == 2*level
                        r2m4 = qpool.tile([128, 512], F32, name="r2m4", tag="r2m4")
                        nc.vector.scalar_tensor_tensor(
                            r2m4[:], e[:], F32_M_MAGIC + 4.0, b2[:], op0=ALU.subtract, op1=ALU.add
                        )
                        # xq = r2m4 * (amax/12) * sign   (on gpsimd to unload DVE)
                        xqm = qpool.tile([128, 512], F32, name="xqm", tag="xqm")
                        nc.vector.tensor_tensor(
                            xqm[:].rearrange("p (g s) -> p g s", s=16),
                            r2m4[:].rearrange("p (g s) -> p g s", s=16),
                            s12_bc, op=ALU.mult,
                        )
                        xqn = qpool.tile([128, 512], BF16, name="xqn", tag="xqn")
                        nc.gpsimd.tensor_tensor(xqn[:], xqm[:], sgn[:], op=ALU.mult)
                        # transpose quantized chunk into per-block xqt tiles
                        for jj in range(4):
                            b = 4 * c + jj
                            qps = xqtps_pool.tile([128, 128], BF16, name="qps", tag="qps")
                            nc.tensor.transpose(
                                qps[:], xqn[:, jj * 128 : (jj + 1) * 128], ident_b[:]
                            )
                            if b % 2 == 0:
                                nc.scalar.copy(xqt[b][:, m * 128 : (m + 1) * 128], qps[:])
                            else:
                                nc.vector.tensor_copy(
                                    xqt[b][:, m * 128 : (m + 1) * 128], qps[:]
                                )

            # ---------------- MAIN: out = xq @ W^T + bias -------------------
            with (
                tc.tile_pool(name="wload", bufs=2) as wpool,
                tc.tile_pool(name="wt_ps", bufs=4, space="PSUM") as wtps_pool,
                tc.tile_pool(name="wt_sb", bufs=5) as wts_pool,
                tc.tile_pool(name="acc", bufs=1, space="PSUM") as acc_pool,
                tc.tile_pool(name="ot", bufs=3) as opool,
            ):
                for n in range(NN):
                    wn = []
                    for half in range(2):
                        for sub in range(4):
                            wsb = wpool.tile(
                                [128, KH], BF16, name=f"wsb{sub}_{half}", tag=f"w{sub}{half}"
                            )
                            nc.gpsimd.dma_start(
                                wsb[:],
                                w_d[
                                    n * 512 + sub * 128 : n * 512 + (sub + 1) * 128,
                                    half * KH : (half + 1) * KH,
                                ],
                            )
                            wn.append(wsb)
                    accs = [
                        acc_pool.tile([128, 512], F32, name=f"acc{m}", tag=f"acc{m}")
                        for m in range(NM)
                    ]
                    for b in range(NB):
                        half, brel = (0, b) if b < NB // 2 else (1, b - NB // 2)
                        wtp = wtps_pool.tile([128, 512], BF16, name="wtp", tag="wtp")
                        for sub in range(4):
                            nc.tensor.transpose(
                                wtp[:, sub * 128 : (sub + 1) * 128],
                                wn[half * 4 + sub][:, brel * HB : (brel + 1) * HB],
                                ident_b[:],
                            )
                        wts = wts_pool.tile([128, 512], BF16, name="wts", tag="wts")
                        if b % 2 == 0:
                            nc.scalar.copy(wts[:], wtp[:])
                        else:
                            nc.vector.tensor_copy(wts[:], wtp[:])
                        for m in range(NM):
                            nc.tensor.matmul(
                                accs[m][:],
                                xqt[b][:, m * 128 : (m + 1) * 128],
                                wts[:],
                                start=(b == 0),
                                stop=(b == NB - 1),
                            )
                    for m in range(NM):
                        ot = opool.tile([128, 512], F32, name="ot", tag="ot")
                        nc.vector.tensor_tensor(
                            ot[:], accs[m][:], bt[:, n * 512 : (n + 1) * 512], op=ALU.add
                        )
                        nc.sync.dma_start(
                            o_d[m * 128 : (m + 1) * 128, n * 512 : (n + 1) * 512], ot[:]
                        )

        for _rep in range(repeat):
            _emit_body()

    nc.compile()
    return nc


def _get_program(rows, din, dout):
    key = (rows, din, dout)
    if key not in _CACHE:
        _CACHE[key] = _build(rows, din, dout)
    return _CACHE[key]


def kernel(x, smooth_scale, S_in, H_block, w_quantized, bias):
    from concourse import bass_utils

    B, S, DIN = x.shape
    DOUT = w_quantized.shape[0]
    n_cores = 8
    rows_total = B * S
    rows = rows_total // n_cores

    nc = _get_program(rows, DIN, DOUT)

    xf = np.ascontiguousarray(x.reshape(rows_total, DIN), dtype=np.float32)
    ss = np.ascontiguousarray(smooth_scale, dtype=np.float32)
    si = np.ascontiguousarray(S_in, dtype=np.float32)
    hb = np.ascontiguousarray(H_block, dtype=np.float32)
    w = np.ascontiguousarray(w_quantized, dtype=np.float32)
    bs = np.ascontiguousarray(bias, dtype=np.float32)

    in_maps = []
    for i in range(n_cores):
        in_maps.append(
            {
                "x": xf[i * rows : (i + 1) * rows],
                "smooth": ss,
                "sin": si,
                "hb": hb,
                "w": w,
                "bias": bs,
            }
        )
    res = bass_utils.run_bass_kernel_spmd(nc, in_maps, core_ids=list(range(n_cores)))
    out = np.concatenate([r["out"] for r in res.results], axis=0)
    return out.reshape(B, S, DOUT).astype(np.float32)

